# revision 37
# baseline (speedup 1.0000x reference)
"""Causal self-attention (B=4, T=1024, C=2048, H=16, rotary) on 8 trn2 cores.

Sharding: core c = 2*b + g handles batch b, head-group g (heads 8g..8g+7).
 - QKV projection computed in transposed layout: Q^T/K^T = [d_channels, T],
   V in natural [T, d_channels] layout (for the att@V contraction).
 - RoPE via host-precomputed full-height cos/sin tables; the rotate-half
   partition swap runs on the PE as a permutation matmul.
 - Scores computed transposed S^T = K_tile^T . Q -> [k, q]; softmax without
   max-subtraction (logits are ~N(0,1); exp can't overflow); causal masking
   via multiplicative 0/1 bf16 masks on diagonal-straddling blocks only;
   denominator via ones-vector matmul accumulated over k-tiles.
 - att@V accumulated in PSUM over k-tiles -> y^T [d, q]; normalized by
   reciprocal row-sums broadcast across partitions via a 1-partition
   PE matmul (ones[1,128]^T @ recip[1,512]).
 - Staged AllGather (pairs sharing a batch): TUNE["stage_heads"] splits the
   8 heads into gather stages fired as soon as their heads are normalized,
   so collective + reload latency hide behind the remaining attention;
   c_proj runs one pass per stage, accumulating into an SBUF bf16 tile,
   with the final pass adding the carried partials and storing.
 - A short PE warm-up matmul chain ramps the p-state while the first
   weight/activation DMAs land.
All matmuls in bf16 (fp32 PSUM accumulation), bf16 output stores.
"""

import math

import numpy as np
import ml_dtypes

BF16 = ml_dtypes.bfloat16

B, T, C = 4, 1024, 2048
H = 16  # total heads
D = C // H  # 128 head dim
HG = 8  # heads per group (per core)
N_CORES = 8
ROPE_BASE = 10000.0

TUNE = {
    "chunk_order": (0, 2, 4, 1, 3, 5),
    "ps_bufs": 2,
    "ps_s_bufs": 3,
    "ps_y_bufs": 2,
    "p_sb_bufs": 6,
    "warmup_mms": 20,
    "pend_kt": 2,
    "stage_heads": (2, 4, 2),
}

PK1_COLS = 1024 + 1024 + 128  # cos | sin | perm
PK2_COLS = 128 + 1024 + 1024  # tri-mask | bv | bp


_PROGRAM_CACHE = {}


def _build_program(num_devices=N_CORES, collective=True, reps=1):
    import concourse.mybir as mybir
    import concourse.tile as tile
    from concourse import bacc
    from concourse.bass import ts

    f32 = mybir.dt.float32
    bf16 = mybir.dt.bfloat16
    AF = mybir.ActivationFunctionType

    nc = bacc.Bacc(trn_type="TRN2", num_devices=num_devices, debug=False)

    # ---- per-core I/O ----
    xT = nc.dram_tensor("xT", [C, T], bf16, kind="ExternalInput")  # x[b].T
    wqkv = nc.dram_tensor("wqkv", [C, 3 * HG * D], bf16, kind="ExternalInput")
    pk1 = nc.dram_tensor("pk1", [128, PK1_COLS], bf16, kind="ExternalInput")
    bqk = nc.dram_tensor("bqk", [128, 16], f32, kind="ExternalInput")
    pk2 = nc.dram_tensor("pk2", [128, PK2_COLS], bf16, kind="ExternalInput")
    wproj = nc.dram_tensor("wproj", [C, C // 2], bf16, kind="ExternalInput")
    out = nc.dram_tensor("out", [T, C // 2], bf16, kind="ExternalOutput")

    xT_r = xT.ap().rearrange("(ct p) t -> p ct t", p=128)  # [128, 16, 1024]
    wqkv_r = wqkv.ap().rearrange("(ct p) j -> p ct j", p=128)  # [128, 16, 3072]
    wproj_r = wproj.ap().rearrange("(jt p) c -> p jt c", p=128)  # [128, 16, 1024]

    scale = 1.0 / math.sqrt(D)

    with tile.TileContext(nc) as tc:
        with (
            tc.tile_pool(name="const", bufs=1) as const,
            tc.tile_pool(name="persist", bufs=1) as persist,
            tc.tile_pool(name="ps", bufs=TUNE["ps_bufs"], space="PSUM") as pspool,
            tc.tile_pool(
                name="ps_s", bufs=TUNE["ps_s_bufs"], space="PSUM"
            ) as ps_s_pool,
            tc.tile_pool(
                name="ps_y", bufs=TUNE["ps_y_bufs"], space="PSUM"
            ) as ps_y_pool,
            tc.tile_pool(name="ps_sum", bufs=1, space="PSUM") as ps_sum_pool,
            tc.tile_pool(name="work", bufs=4) as work,
            tc.tile_pool(name="dram", bufs=1, space="DRAM") as drampool,
        ):
            # ---- constants (tiles created here; DMAs emitted in phase A
            # after the first weight chunk + xs so startup matmuls begin
            # as early as possible) ----
            pk1_sb = const.tile([128, PK1_COLS], bf16)
            pk2_sb = const.tile([128, PK2_COLS], bf16)
            cos_sb = pk1_sb[:, 0:1024]
            sin_sb = pk1_sb[:, 1024:2048]
            perm_sb = pk1_sb[:, 2048:2176]
            bqk_sb = const.tile([128, 16], f32)
            bv_bc = pk2_sb[:, 128:1152]
            bp_bc = pk2_sb[:, 1152:2176]
            ones_sb = const.tile([128, 1], bf16)
            nc.vector.memset(ones_sb, 1.0)
            ones_row = const.tile([1, 128], bf16)
            nc.vector.memset(ones_row, 1.0)

            # ---- persistent activations (reused across reps) ----
            qf = persist.tile([128, HG, T], bf16)  # [d, h, t] rotated Q^T
            kf = persist.tile([128, HG, T], bf16)  # [d, h, t] rotated K^T
            v_all = persist.tile([128, 8, HG * D], bf16)  # [t_in, tt, j]
            yT = persist.tile([128, HG, T], bf16)  # [d, h, t] normalized att out

            for rep in range(reps):
                _emit_once(
                    nc, tc, mybir, ts, f32, bf16, AF, scale, collective, rep,
                    xT_r, wqkv_r, wproj_r, out, pk1, pk2, bqk, pk1_sb, pk2_sb,
                    cos_sb, sin_sb, bqk_sb, perm_sb, bv_bc, bp_bc,
                    ones_sb, ones_row,
                    qf, kf, v_all, yT,
                    pspool, ps_s_pool, ps_y_pool, ps_sum_pool, work, drampool,
                )

    nc.finalize()
    return nc


def _emit_once(
    nc, tc, mybir, ts, f32, bf16, AF, scale, collective, rep,
    xT_r, wqkv_r, wproj_r, out, pk1, pk2, bqk, pk1_sb, pk2_sb,
    cos_sb, sin_sb, bqk_sb, perm_sb, bv_bc, bp_bc, ones_sb, ones_row,
    qf, kf, v_all, yT,
    pspool, ps_s_pool, ps_y_pool, ps_sum_pool, work, drampool,
):
    mask_sb = pk2_sb  # columns [0:128] = lower-triangular diagonal-block mask

    # shared PSUM bank: partitions 0/32 hold alternating softmax-denominator
    # slices; partition 64 is a junk target for PE warm-up matmuls
    sum_bank = ps_sum_pool.tile(
        [128, 512], f32, tag="sum_bank", name=f"sum_bank{rep}"
    )
    if rep == 0 and TUNE["warmup_mms"]:
        # keep the PE busy (and its p-state ramping) while the first
        # weight/activation DMAs land
        warm = work.tile([128, 512], bf16, tag="warm", name="warm")
        nc.vector.memset(warm, 0.0)
        n_warm = TUNE["warmup_mms"]
        for i in range(n_warm):
            nc.tensor.matmul(
                sum_bank[64:65, :], lhsT=ones_sb, rhs=warm,
                start=(i == 0), stop=(i == n_warm - 1),
            )

    # =========== Phase A: QKV projection (+bias, +RoPE) ===========
    with (
        tc.tile_pool(name=f"xpool{rep}", bufs=1) as xpool,
        tc.tile_pool(name=f"wpool{rep}", bufs=2) as wpool,
    ):
        xs = xpool.tile([128, 16, T], bf16, name="xs")

        # order q0,k0,v0 first so heads 0-3 complete early and their
        # attention overlaps the rest of the QKV projection
        for chunk_i, chunk in enumerate(TUNE["chunk_order"]):
            wt = wpool.tile([128, 16, 512], bf16, tag="wt", name="wt")
            c0 = chunk * 512
            if chunk_i == 0:
                # startup DMA order: wt0 j-half, xs t-half 0, wt0 j-half 2,
                # consts, xs t-half 1 — the first QK tiles (th=0, jj=0,1)
                # need only the first halves
                nc.sync.dma_start(
                    out=wt[:, :, 0:256], in_=wqkv_r[:, :, c0 : c0 + 256]
                )
                nc.sync.dma_start(out=xs[:, :, 0:512], in_=xT_r[:, :, 0:512])
                nc.sync.dma_start(
                    out=wt[:, :, 256:512], in_=wqkv_r[:, :, c0 + 256 : c0 + 512]
                )
                if rep == 0:
                    nc.sync.dma_start(out=bqk_sb, in_=bqk.ap())
                    nc.sync.dma_start(out=pk1_sb, in_=pk1.ap())
                nc.sync.dma_start(out=xs[:, :, 512:1024], in_=xT_r[:, :, 512:1024])
                if rep == 0:
                    nc.sync.dma_start(out=pk2_sb, in_=pk2.ap())
            else:
                nc.sync.dma_start(out=wt, in_=wqkv_r[:, :, c0 : c0 + 512])
            if chunk < 4:  # Q or K, output transposed [j, t]
                for th in range(2):  # t halves of 512 (th outer: xs half 0 first)
                    for jj in range(4):
                        jt = chunk * 4 + jj  # 0..15 (q: 0-7, k: 8-15)
                        h = jt % 8
                        dest_all = qf if jt < 8 else kf
                        ps = pspool.tile([128, 512], f32, tag="ps", name="ps")
                        for ct in range(16):
                            nc.tensor.matmul(
                                ps,
                                lhsT=wt[:, ct, jj * 128 : (jj + 1) * 128],
                                rhs=xs[:, ct, ts(th, 512)],
                                start=(ct == 0),
                                stop=(ct == 15),
                            )
                        raw = work.tile([128, 512], bf16, tag="raw", name="raw")
                        nc.vector.tensor_scalar_add(
                            out=raw, in0=ps, scalar1=bqk_sb[:, jt : jt + 1]
                        )
                        # RoPE: out = raw*cos2 + swap_halves(raw)*sin2
                        # half-swap on PE via permutation matmul (DVE
                        # can't move data across partitions)
                        dest = dest_all[:, h, ts(th, 512)]
                        ps_swp = ps_s_pool.tile(
                            [128, 512], f32, tag="ps_sc", name="ps_swp"
                        )
                        nc.tensor.matmul(
                            ps_swp, lhsT=perm_sb, rhs=raw, start=True, stop=True
                        )
                        rtmp = work.tile([128, 512], bf16, tag="rtmp", name="rtmp")
                        nc.vector.tensor_mul(rtmp, ps_swp, sin_sb[:, ts(th, 512)])
                        nc.vector.tensor_mul(dest, raw, cos_sb[:, ts(th, 512)])
                        nc.vector.tensor_add(dest, dest, rtmp)
            else:  # V, natural layout [t, j]
                jc = chunk - 4  # 0 or 1
                for tt in range(8):
                    ps = pspool.tile([128, 512], f32, tag="ps", name="ps")
                    for ct in range(16):
                        nc.tensor.matmul(
                            ps,
                            lhsT=xs[:, ct, ts(tt, 128)],
                            rhs=wt[:, ct, :],
                            start=(ct == 0),
                            stop=(ct == 15),
                        )
                    nc.vector.tensor_add(
                        v_all[:, tt, jc * 512 : (jc + 1) * 512],
                        ps,
                        bv_bc[:, jc * 512 : (jc + 1) * 512],
                    )

    # proj weights: load now so the DMA overlaps phase B
    projpool = tc.tile_pool(name=f"proj{rep}", bufs=1)
    proj = projpool.__enter__()
    try:
        wp = proj.tile([128, 16, C // 2], bf16, name="wp")
        nc.sync.dma_start(out=wp, in_=wproj_r)

        # DRAM bounce + gathered tensors, one set per gather stage
        stage_heads = TUNE["stage_heads"]
        stages = []  # (h0, h1, yb, yg, ygs)
        h0 = 0
        for si, nh in enumerate(stage_heads):
            h1 = h0 + nh
            yb = drampool.tile([nh * D, T], bf16, name=f"yb{si}")
            yg = drampool.tile([2 * nh * D, T], bf16, name=f"yg{si}")
            ygs = proj.tile([128, 2 * nh, T], bf16, name=f"ygs{si}")
            stages.append((h0, h1, yb, yg, ygs))
            h0 = h1
        assert h0 == HG
        acc = proj.tile([128, 16, 512], bf16, name="acc")

        # =========== Phase B: attention per (head, q-chunk) ===========
        # The normalize of group g is emitted after group g+1's first
        # k-tile so the PE's broadcast matmul never stalls the queue on
        # the reciprocal chain. Alternating groups use disjoint [1,512]
        # slices (partitions 0/32) of the shared PSUM bank.
        pending = None

        def emit_normalize(pn):
            ph, pqc, p_ps_y, p_ps_sum = pn
            recip_b = work.tile([1, 512], bf16, tag="recip_b", name="recip_b")
            with nc.allow_low_precision("softmax recip scale, bf16 is plenty"):
                nc.vector.reciprocal(recip_b, p_ps_sum)
            # broadcast across partitions on PE: ones[1,128]^T @ r[1,512]
            ps_bc = pspool.tile([128, 512], f32, tag="ps", name="ps_bc")
            nc.tensor.matmul(
                ps_bc, lhsT=ones_row, rhs=recip_b, start=True, stop=True
            )
            rb = work.tile([128, 512], bf16, tag="rb", name="rb")
            nc.scalar.activation(rb, ps_bc, AF.Copy)
            nc.vector.tensor_mul(yT[:, ph, ts(pqc, 512)], p_ps_y, rb)

        for h in range(HG):
            for qc in range(2):  # q chunks of 512
                n_kt = 4 * (qc + 1)  # causal: valid k tiles
                ps_y = ps_y_pool.tile([128, 512], f32, tag="ps_y", name="ps_y")
                g_par = 32 * ((2 * h + qc) % 2)
                ps_sum = sum_bank[g_par : g_par + 1, :]
                for kt in range(n_kt):
                    # causal: k block kt only sees q >= kt*128, so compute
                    # just the valid column range [off, 512) of this chunk
                    kt_rel = kt - 4 * qc
                    off = max(0, kt_rel) * 128
                    ps_sc = ps_s_pool.tile(
                        [128, 512], f32, tag="ps_sc", name="ps_sc"
                    )
                    nc.tensor.matmul(
                        ps_sc[:, off:512],
                        lhsT=kf[:, h, ts(kt, 128)],
                        rhs=qf[:, h, qc * 512 + off : (qc + 1) * 512],
                        start=True,
                        stop=True,
                    )
                    p_sb = work.tile(
                        [128, 512], bf16, tag="p_sb", name="p_sb",
                        bufs=TUNE["p_sb_bufs"],
                    )
                    nc.scalar.activation(
                        p_sb[:, off:512], ps_sc[:, off:512], AF.Exp, scale=scale
                    )
                    if kt_rel >= 0:  # triangle mask on the diagonal block
                        nc.vector.tensor_mul(
                            p_sb[:, off : off + 128],
                            p_sb[:, off : off + 128],
                            mask_sb[:, 0:128],
                        )
                    nc.tensor.matmul(
                        ps_sum[:, off:512],
                        lhsT=ones_sb,
                        rhs=p_sb[:, off:512],
                        start=(kt == 0),
                        stop=(kt == n_kt - 1),
                    )
                    nc.tensor.matmul(
                        ps_y[:, off:512],
                        lhsT=v_all[:, kt, ts(h, 128)],
                        rhs=p_sb[:, off:512],
                        start=(kt == 0),
                        stop=(kt == n_kt - 1),
                    )
                    if kt == TUNE["pend_kt"] and pending is not None:
                        emit_normalize(pending)
                        pending = None
                pending = (h, qc, ps_y, ps_sum)

            for si, (sh0, sh1, yb, yg, ygs) in enumerate(stages):
                if h == sh1 - 1:  # stage gather while later heads compute
                    if pending is not None:
                        emit_normalize(pending)
                        pending = None
                    nh = sh1 - sh0
                    yb_r = yb.rearrange("(h p) t -> p h t", p=128)
                    nc.sync.dma_start(out=yb_r, in_=yT[:, sh0:sh1, :])
                    if collective:
                        nc.gpsimd.collective_compute(
                            "AllGather",
                            mybir.AluOpType.bypass,
                            replica_groups=[[0, 1], [2, 3], [4, 5], [6, 7]],
                            ins=[yb[:].opt()],
                            outs=[yg[:].opt()],
                        )
                    else:  # timeline-sim variant: fake the gather locally
                        nc.sync.dma_start(out=yg[0 : nh * D, :], in_=yb[:])
                        nc.sync.dma_start(out=yg[nh * D :, :], in_=yb[:])
                    nc.sync.dma_start(
                        out=ygs, in_=yg.rearrange("(jt p) t -> p jt t", p=128)
                    )

        # =========== Phase C: c_proj, one pass per gather stage ===========
        n_stages = len(stages)
        for si, (sh0, sh1, yb, yg, ygs) in enumerate(stages):
            jts = list(range(sh0, sh1)) + list(range(8 + sh0, 8 + sh1))
            for tt in range(8):
                o_sb = None
                for cc in range(2):  # output col chunks of 512
                    ps = pspool.tile([128, 512], f32, tag="ps", name="ps_proj")
                    for i, jt in enumerate(jts):
                        nc.tensor.matmul(
                            ps,
                            lhsT=ygs[:, i, ts(tt, 128)],
                            rhs=wp[:, jt, ts(cc, 512)],
                            start=(i == 0),
                            stop=(i == len(jts) - 1),
                        )
                    a_sl = acc[:, tt * 2 + cc, :]
                    if si == 0:
                        nc.vector.tensor_add(a_sl, ps, bp_bc[:, ts(cc, 512)])
                    elif si < n_stages - 1:
                        nc.vector.tensor_add(a_sl, ps, a_sl)
                    else:
                        if o_sb is None:
                            o_sb = work.tile(
                                [128, 1024], bf16, tag="o_sb", name="o_sb"
                            )
                        nc.vector.tensor_add(o_sb[:, ts(cc, 512)], ps, a_sl)
                if o_sb is not None:  # one store per 128-row band
                    nc.sync.dma_start(out=out.ap()[ts(tt, 128), :], in_=o_sb)
    finally:
        projpool.__exit__(None, None, None)


def _host_inputs(x, w_attn, b_attn, w_proj, b_proj):
    """Build the 8 per-core input maps."""
    x = np.asarray(x, np.float32)
    w_attn = np.asarray(w_attn, np.float32)
    b_attn = np.asarray(b_attn, np.float32)
    w_proj = np.asarray(w_proj, np.float32)
    b_proj = np.asarray(b_proj, np.float32)

    # rope tables, transposed [d, t], full height with rotate-half signs folded:
    # out = x * cos2 + swap_halves(x) * sin2,  cos2=[cos;cos], sin2=[-sin;sin]
    inv_freq = 1.0 / (ROPE_BASE ** (np.arange(0, D, 2, dtype=np.float32) / D))
    freqs = np.arange(T, dtype=np.float32)[:, None] * inv_freq[None, :]  # [T, 64]
    c_ = np.ascontiguousarray(np.cos(freqs).T)  # [64, T]
    s_ = np.ascontiguousarray(np.sin(freqs).T)
    cosT = np.concatenate([c_, c_], axis=0).astype(np.float32)  # [128, T]
    sinT = np.concatenate([-s_, s_], axis=0).astype(np.float32)

    # single lower-triangular [k, q] mask for the diagonal 128x128 block
    k_idx = np.arange(128)
    maskT = (k_idx[:, None] <= k_idx[None, :]).astype(np.float32)

    permM = np.zeros((128, 128), np.float32)
    permM[(np.arange(128) + 64) % 128, np.arange(128)] = 1.0

    in_maps = []
    for c in range(N_CORES):
        b, g = divmod(c, 2)
        cs = slice(g * 1024, (g + 1) * 1024)
        wq = w_attn[:, 0:C][:, cs]
        wk = w_attn[:, C : 2 * C][:, cs]
        wv = w_attn[:, 2 * C : 3 * C][:, cs]
        bq = b_attn[0:C][cs]
        bk = b_attn[C : 2 * C][cs]
        bvv = b_attn[2 * C : 3 * C][cs]
        bqk = np.ascontiguousarray(np.concatenate([bq, bk]).reshape(16, 128).T)
        pk1 = np.concatenate([cosT, sinT, permM], axis=1)  # [128, 2176]
        pk2 = np.concatenate(
            [
                maskT,
                np.broadcast_to(bvv.reshape(1, 1024), (128, 1024)),
                np.broadcast_to(b_proj[cs].reshape(1, 1024), (128, 1024)),
            ],
            axis=1,
        )  # [128, 4096]
        in_maps.append(
            {
                "xT": np.ascontiguousarray(x[b].T).astype(BF16),
                "wqkv": np.concatenate([wq, wk, wv], axis=1).astype(BF16),
                "bqk": bqk.astype(np.float32),
                "pk1": pk1.astype(BF16),
                "pk2": np.ascontiguousarray(pk2).astype(BF16),
                "wproj": w_proj[:, cs].astype(BF16),
            }
        )
    return in_maps


def kernel(x, w_attn, b_attn, w_proj, b_proj, _trace=False):
    from concourse.bass_utils import run_bass_kernel_spmd

    if "nc" not in _PROGRAM_CACHE:
        _PROGRAM_CACHE["nc"] = _build_program()
    nc = _PROGRAM_CACHE["nc"]

    in_maps = _host_inputs(x, w_attn, b_attn, w_proj, b_proj)
    res = run_bass_kernel_spmd(
        nc, in_maps, core_ids=list(range(N_CORES)), trace=_trace
    )
    _PROGRAM_CACHE["last_results"] = res

    out = np.zeros((B, T, C), np.float32)
    for c in range(N_CORES):
        b, g = divmod(c, 2)
        out[b, :, g * 1024 : (g + 1) * 1024] = res.results[c]["out"]
    return out


# revision 40
# speedup vs baseline: 1.2108x; 1.2108x over previous
"""Causal self-attention (B=4, T=1024, C=2048, H=16, rotary) on 8 trn2 cores.

Sharding: core c = 2*b + g handles batch b, head-group g (heads 8g..8g+7).
 - QKV projection computed in transposed layout: Q^T/K^T = [d_channels, T],
   V in natural [T, d_channels] layout (for the att@V contraction).
 - RoPE via host-precomputed full-height cos/sin tables; the rotate-half
   partition swap runs on the PE as a permutation matmul.
 - Scores computed transposed S^T = K_tile^T . Q -> [k, q]; softmax without
   max-subtraction (logits are ~N(0,1); exp can't overflow); causal masking
   via multiplicative 0/1 bf16 masks on diagonal-straddling blocks only;
   denominator via ones-vector matmul accumulated over k-tiles.
 - att@V accumulated in PSUM over k-tiles -> y^T [d, q]; normalized by
   reciprocal row-sums broadcast across partitions via a 1-partition
   PE matmul (ones[1,128]^T @ recip[1,512]).
 - Staged AllGather (pairs sharing a batch): TUNE["stage_heads"] splits the
   8 heads into gather stages fired as soon as their heads are normalized,
   so collective + reload latency hide behind the remaining attention;
   c_proj runs one pass per stage, accumulating into an SBUF bf16 tile,
   with the final pass adding the carried partials and storing.
 - A short PE warm-up matmul chain ramps the p-state while the first
   weight/activation DMAs land.
All matmuls in bf16 (fp32 PSUM accumulation), bf16 output stores.
"""

import math

import numpy as np
import ml_dtypes

BF16 = ml_dtypes.bfloat16

B, T, C = 4, 1024, 2048
H = 16  # total heads
D = C // H  # 128 head dim
HG = 8  # heads per group (per core)
N_CORES = 8
ROPE_BASE = 10000.0

TUNE = {
    "chunk_order": (0, 2, 4, 1, 3, 5),
    "ps_bufs": 2,
    "ps_s_bufs": 3,
    "ps_y_bufs": 2,
    "p_sb_bufs": 6,
    "warmup_mms": 26,
    "pend_kt": 2,
    "stage_heads": (2, 4, 2),
}

PK1_COLS = 1024 + 1024 + 128  # cos | sin | perm
PK2_COLS = 128 + 1024 + 1024  # tri-mask | bv | bp


_PROGRAM_CACHE = {}


def _build_program(num_devices=N_CORES, collective=True, reps=1):
    import concourse.mybir as mybir
    import concourse.tile as tile
    from concourse import bacc
    from concourse.bass import ts

    f32 = mybir.dt.float32
    bf16 = mybir.dt.bfloat16
    AF = mybir.ActivationFunctionType

    nc = bacc.Bacc(trn_type="TRN2", num_devices=num_devices, debug=False)

    # ---- per-core I/O ----
    xT = nc.dram_tensor("xT", [C, T], bf16, kind="ExternalInput")  # x[b].T
    wqkv = nc.dram_tensor("wqkv", [C, 3 * HG * D], bf16, kind="ExternalInput")
    pk1 = nc.dram_tensor("pk1", [128, PK1_COLS], bf16, kind="ExternalInput")
    bqk = nc.dram_tensor("bqk", [128, 16], f32, kind="ExternalInput")
    pk2 = nc.dram_tensor("pk2", [128, PK2_COLS], bf16, kind="ExternalInput")
    wproj = nc.dram_tensor("wproj", [C, C // 2], bf16, kind="ExternalInput")
    out = nc.dram_tensor("out", [T, C // 2], bf16, kind="ExternalOutput")

    xT_r = xT.ap().rearrange("(ct p) t -> p ct t", p=128)  # [128, 16, 1024]
    wqkv_r = wqkv.ap().rearrange("(ct p) j -> p ct j", p=128)  # [128, 16, 3072]
    wproj_r = wproj.ap().rearrange("(jt p) c -> p jt c", p=128)  # [128, 16, 1024]

    scale = 1.0 / math.sqrt(D)

    with tile.TileContext(nc) as tc:
        with (
            tc.tile_pool(name="const", bufs=1) as const,
            tc.tile_pool(name="persist", bufs=1) as persist,
            tc.tile_pool(name="ps", bufs=TUNE["ps_bufs"], space="PSUM") as pspool,
            tc.tile_pool(
                name="ps_s", bufs=TUNE["ps_s_bufs"], space="PSUM"
            ) as ps_s_pool,
            tc.tile_pool(
                name="ps_y", bufs=TUNE["ps_y_bufs"], space="PSUM"
            ) as ps_y_pool,
            tc.tile_pool(name="ps_sum", bufs=1, space="PSUM") as ps_sum_pool,
            tc.tile_pool(name="work", bufs=4) as work,
            tc.tile_pool(name="dram", bufs=1, space="DRAM") as drampool,
        ):
            # ---- constants (tiles created here; DMAs emitted in phase A
            # after the first weight chunk + xs so startup matmuls begin
            # as early as possible) ----
            pk1_sb = const.tile([128, PK1_COLS], bf16)
            pk2_sb = const.tile([128, PK2_COLS], bf16)
            cos_sb = pk1_sb[:, 0:1024]
            sin_sb = pk1_sb[:, 1024:2048]
            perm_sb = pk1_sb[:, 2048:2176]
            bqk_sb = const.tile([128, 16], f32)
            bv_bc = pk2_sb[:, 128:1152]
            bp_bc = pk2_sb[:, 1152:2176]
            ones_sb = const.tile([128, 1], bf16)
            nc.vector.memset(ones_sb, 1.0)
            ones_row = const.tile([1, 128], bf16)
            nc.vector.memset(ones_row, 1.0)

            # ---- persistent activations (reused across reps) ----
            qf = persist.tile([128, HG, T], bf16)  # [d, h, t] rotated Q^T
            kf = persist.tile([128, HG, T], bf16)  # [d, h, t] rotated K^T
            v_all = persist.tile([128, 8, HG * D], bf16)  # [t_in, tt, j]
            yT = persist.tile([128, HG, T], bf16)  # [d, h, t] normalized att out

            for rep in range(reps):
                _emit_once(
                    nc, tc, mybir, ts, f32, bf16, AF, scale, collective, rep,
                    xT_r, wqkv_r, wproj_r, out, pk1, pk2, bqk, pk1_sb, pk2_sb,
                    cos_sb, sin_sb, bqk_sb, perm_sb, bv_bc, bp_bc,
                    ones_sb, ones_row,
                    qf, kf, v_all, yT,
                    pspool, ps_s_pool, ps_y_pool, ps_sum_pool, work, drampool,
                )

    nc.finalize()
    return nc


def _emit_once(
    nc, tc, mybir, ts, f32, bf16, AF, scale, collective, rep,
    xT_r, wqkv_r, wproj_r, out, pk1, pk2, bqk, pk1_sb, pk2_sb,
    cos_sb, sin_sb, bqk_sb, perm_sb, bv_bc, bp_bc, ones_sb, ones_row,
    qf, kf, v_all, yT,
    pspool, ps_s_pool, ps_y_pool, ps_sum_pool, work, drampool,
):
    mask_sb = pk2_sb  # columns [0:128] = lower-triangular diagonal-block mask

    # shared PSUM bank: partitions 0/32 hold alternating softmax-denominator
    # slices; partition 64 is a junk target for PE warm-up matmuls
    sum_bank = ps_sum_pool.tile(
        [128, 512], f32, tag="sum_bank", name=f"sum_bank{rep}"
    )
    if rep == 0 and TUNE["warmup_mms"]:
        # keep the PE busy (and its p-state ramping) while the first
        # weight/activation DMAs land
        warm = work.tile([128, 512], bf16, tag="warm", name="warm")
        nc.vector.memset(warm, 0.0)
        n_warm = TUNE["warmup_mms"]
        for i in range(n_warm):
            nc.tensor.matmul(
                sum_bank[64:65, :], lhsT=ones_sb, rhs=warm,
                start=(i == 0), stop=(i == n_warm - 1),
            )
        # tiny dummy gather: pre-establish the pair's collective path and
        # absorb launch skew during the startup DMA window (content unused)
        dmy_i = drampool.tile([1, 128], bf16, name="dmy_i")
        dmy_o = drampool.tile([2, 128], bf16, name="dmy_o")
        if collective:
            nc.gpsimd.collective_compute(
                "AllGather",
                mybir.AluOpType.bypass,
                replica_groups=[[0, 1], [2, 3], [4, 5], [6, 7]],
                ins=[dmy_i[:].opt()],
                outs=[dmy_o[:].opt()],
            )
        else:
            nc.sync.dma_start(out=dmy_o[0:1, :], in_=dmy_i[:])

    # =========== Phase A: QKV projection (+bias, +RoPE) ===========
    with (
        tc.tile_pool(name=f"xpool{rep}", bufs=1) as xpool,
        tc.tile_pool(name=f"wpool{rep}", bufs=2) as wpool,
    ):
        xs = xpool.tile([128, 16, T], bf16, name="xs")

        # order q0,k0,v0 first so heads 0-3 complete early and their
        # attention overlaps the rest of the QKV projection
        for chunk_i, chunk in enumerate(TUNE["chunk_order"]):
            wt = wpool.tile([128, 16, 512], bf16, tag="wt", name="wt")
            c0 = chunk * 512
            if chunk_i == 0:
                # startup DMA order: wt0 j-half, xs t-half 0, wt0 j-half 2,
                # consts, xs t-half 1 — the first QK tiles (th=0, jj=0,1)
                # need only the first halves
                nc.sync.dma_start(
                    out=wt[:, :, 0:256], in_=wqkv_r[:, :, c0 : c0 + 256]
                )
                nc.sync.dma_start(out=xs[:, :, 0:512], in_=xT_r[:, :, 0:512])
                nc.sync.dma_start(
                    out=wt[:, :, 256:512], in_=wqkv_r[:, :, c0 + 256 : c0 + 512]
                )
                if rep == 0:
                    nc.sync.dma_start(out=bqk_sb, in_=bqk.ap())
                    nc.sync.dma_start(out=pk1_sb, in_=pk1.ap())
                nc.sync.dma_start(out=xs[:, :, 512:1024], in_=xT_r[:, :, 512:1024])
                if rep == 0:
                    nc.sync.dma_start(out=pk2_sb, in_=pk2.ap())
            else:
                nc.sync.dma_start(out=wt, in_=wqkv_r[:, :, c0 : c0 + 512])
            if chunk < 4:  # Q or K, output transposed [j, t]
                for th in range(2):  # t halves of 512 (th outer: xs half 0 first)
                    for jj in range(4):
                        jt = chunk * 4 + jj  # 0..15 (q: 0-7, k: 8-15)
                        h = jt % 8
                        dest_all = qf if jt < 8 else kf
                        ps = pspool.tile([128, 512], f32, tag="ps", name="ps")
                        for ct in range(16):
                            nc.tensor.matmul(
                                ps,
                                lhsT=wt[:, ct, jj * 128 : (jj + 1) * 128],
                                rhs=xs[:, ct, ts(th, 512)],
                                start=(ct == 0),
                                stop=(ct == 15),
                            )
                        raw = work.tile([128, 512], bf16, tag="raw", name="raw")
                        nc.vector.tensor_scalar_add(
                            out=raw, in0=ps, scalar1=bqk_sb[:, jt : jt + 1]
                        )
                        # RoPE: out = raw*cos2 + swap_halves(raw)*sin2
                        # half-swap on PE via permutation matmul (DVE
                        # can't move data across partitions)
                        dest = dest_all[:, h, ts(th, 512)]
                        ps_swp = ps_s_pool.tile(
                            [128, 512], f32, tag="ps_sc", name="ps_swp"
                        )
                        nc.tensor.matmul(
                            ps_swp, lhsT=perm_sb, rhs=raw, start=True, stop=True
                        )
                        rtmp = work.tile([128, 512], bf16, tag="rtmp", name="rtmp")
                        nc.vector.tensor_mul(rtmp, ps_swp, sin_sb[:, ts(th, 512)])
                        nc.vector.tensor_mul(dest, raw, cos_sb[:, ts(th, 512)])
                        nc.vector.tensor_add(dest, dest, rtmp)
            else:  # V, natural layout [t, j]
                jc = chunk - 4  # 0 or 1
                for tt in range(8):
                    ps = pspool.tile([128, 512], f32, tag="ps", name="ps")
                    for ct in range(16):
                        nc.tensor.matmul(
                            ps,
                            lhsT=xs[:, ct, ts(tt, 128)],
                            rhs=wt[:, ct, :],
                            start=(ct == 0),
                            stop=(ct == 15),
                        )
                    nc.vector.tensor_add(
                        v_all[:, tt, jc * 512 : (jc + 1) * 512],
                        ps,
                        bv_bc[:, jc * 512 : (jc + 1) * 512],
                    )

    # proj weights: load now so the DMA overlaps phase B
    projpool = tc.tile_pool(name=f"proj{rep}", bufs=1)
    proj = projpool.__enter__()
    try:
        wp = proj.tile([128, 16, C // 2], bf16, name="wp")
        nc.sync.dma_start(out=wp, in_=wproj_r)

        # DRAM bounce + gathered tensors, one set per gather stage
        stage_heads = TUNE["stage_heads"]
        stages = []  # (h0, h1, yb, yg, ygs)
        h0 = 0
        for si, nh in enumerate(stage_heads):
            h1 = h0 + nh
            yb = drampool.tile([nh * D, T], bf16, name=f"yb{si}")
            yg = drampool.tile([2 * nh * D, T], bf16, name=f"yg{si}")
            ygs = proj.tile([128, 2 * nh, T], bf16, name=f"ygs{si}")
            stages.append((h0, h1, yb, yg, ygs))
            h0 = h1
        assert h0 == HG
        acc = proj.tile([128, 16, 512], bf16, name="acc")

        # =========== Phase B: attention per (head, q-chunk) ===========
        # The normalize of group g is emitted after group g+1's first
        # k-tile so the PE's broadcast matmul never stalls the queue on
        # the reciprocal chain. Alternating groups use disjoint [1,512]
        # slices (partitions 0/32) of the shared PSUM bank.
        pending = None

        def emit_normalize(pn):
            ph, pqc, p_ps_y, p_ps_sum = pn
            recip_b = work.tile([1, 512], bf16, tag="recip_b", name="recip_b")
            with nc.allow_low_precision("softmax recip scale, bf16 is plenty"):
                nc.vector.reciprocal(recip_b, p_ps_sum)
            # broadcast across partitions on PE: ones[1,128]^T @ r[1,512]
            ps_bc = pspool.tile([128, 512], f32, tag="ps", name="ps_bc")
            nc.tensor.matmul(
                ps_bc, lhsT=ones_row, rhs=recip_b, start=True, stop=True
            )
            rb = work.tile([128, 512], bf16, tag="rb", name="rb")
            nc.scalar.activation(rb, ps_bc, AF.Copy)
            nc.vector.tensor_mul(yT[:, ph, ts(pqc, 512)], p_ps_y, rb)

        order = []
        if TUNE.get("interleave_heads"):
            # pair heads within each stage: (h,0),(h+1,0),(h,1),(h+1,1)
            h0 = 0
            for nh in TUNE["stage_heads"]:
                hs = list(range(h0, h0 + nh))
                for i in range(0, len(hs) - 1, 2):
                    a, b = hs[i], hs[i + 1]
                    order += [(a, 0), (b, 0), (a, 1), (b, 1)]
                if len(hs) % 2:
                    order += [(hs[-1], 0), (hs[-1], 1)]
                h0 += nh
        else:
            qcs = (1, 0) if TUNE.get("qc_rev") else (0, 1)
            order = [(h, qc) for h in range(HG) for qc in qcs]
        done = set()
        for h, qc in order:
            if True:  # keep diff small; loop body below unchanged
                n_kt = 4 * (qc + 1)  # causal: valid k tiles
                ps_y = ps_y_pool.tile([128, 512], f32, tag="ps_y", name="ps_y")
                g_par = 32 * ((2 * h + qc) % 2)
                ps_sum = sum_bank[g_par : g_par + 1, :]
                for kt in range(n_kt):
                    # causal: k block kt only sees q >= kt*128, so compute
                    # just the valid column range [off, 512) of this chunk
                    kt_rel = kt - 4 * qc
                    off = max(0, kt_rel) * 128
                    ps_sc = ps_s_pool.tile(
                        [128, 512], f32, tag="ps_sc", name="ps_sc"
                    )
                    nc.tensor.matmul(
                        ps_sc[:, off:512],
                        lhsT=kf[:, h, ts(kt, 128)],
                        rhs=qf[:, h, qc * 512 + off : (qc + 1) * 512],
                        start=True,
                        stop=True,
                    )
                    p_sb = work.tile(
                        [128, 512], bf16, tag="p_sb", name="p_sb",
                        bufs=TUNE["p_sb_bufs"],
                    )
                    nc.scalar.activation(
                        p_sb[:, off:512], ps_sc[:, off:512], AF.Exp, scale=scale
                    )
                    if kt_rel >= 0:  # triangle mask on the diagonal block
                        nc.vector.tensor_mul(
                            p_sb[:, off : off + 128],
                            p_sb[:, off : off + 128],
                            mask_sb[:, 0:128],
                        )
                    nc.tensor.matmul(
                        ps_sum[:, off:512],
                        lhsT=ones_sb,
                        rhs=p_sb[:, off:512],
                        start=(kt == 0),
                        stop=(kt == n_kt - 1),
                    )
                    nc.tensor.matmul(
                        ps_y[:, off:512],
                        lhsT=v_all[:, kt, ts(h, 128)],
                        rhs=p_sb[:, off:512],
                        start=(kt == 0),
                        stop=(kt == n_kt - 1),
                    )
                    if kt == TUNE["pend_kt"] and pending is not None:
                        emit_normalize(pending)
                        pending = None
                pending = (h, qc, ps_y, ps_sum)

            done.add((h, qc))
            for si, (sh0, sh1, yb, yg, ygs) in enumerate(stages):
                need = {(hh, qq) for hh in range(sh0, sh1) for qq in (0, 1)}
                if (h, qc) in need and need <= done:  # stage fully emitted
                    if pending is not None:
                        emit_normalize(pending)
                        pending = None
                    nh = sh1 - sh0
                    yb_r = yb.rearrange("(h p) t -> p h t", p=128)
                    nc.sync.dma_start(out=yb_r, in_=yT[:, sh0:sh1, :])
                    if collective:
                        nc.gpsimd.collective_compute(
                            "AllGather",
                            mybir.AluOpType.bypass,
                            replica_groups=[[0, 1], [2, 3], [4, 5], [6, 7]],
                            ins=[yb[:].opt()],
                            outs=[yg[:].opt()],
                        )
                    else:  # timeline-sim variant: fake the gather locally
                        nc.sync.dma_start(out=yg[0 : nh * D, :], in_=yb[:])
                        nc.sync.dma_start(out=yg[nh * D :, :], in_=yb[:])
                    nc.sync.dma_start(
                        out=ygs, in_=yg.rearrange("(jt p) t -> p jt t", p=128)
                    )

        # =========== Phase C: c_proj, one pass per gather stage ===========
        n_stages = len(stages)
        for si, (sh0, sh1, yb, yg, ygs) in enumerate(stages):
            jts = list(range(sh0, sh1)) + list(range(8 + sh0, 8 + sh1))
            for tt in range(8):
                o_sb = None
                for cc in range(2):  # output col chunks of 512
                    ps = pspool.tile([128, 512], f32, tag="ps", name="ps_proj")
                    for i, jt in enumerate(jts):
                        nc.tensor.matmul(
                            ps,
                            lhsT=ygs[:, i, ts(tt, 128)],
                            rhs=wp[:, jt, ts(cc, 512)],
                            start=(i == 0),
                            stop=(i == len(jts) - 1),
                        )
                    a_sl = acc[:, tt * 2 + cc, :]
                    if si == 0:
                        nc.vector.tensor_add(a_sl, ps, bp_bc[:, ts(cc, 512)])
                    elif si < n_stages - 1:
                        nc.vector.tensor_add(a_sl, ps, a_sl)
                    else:
                        if o_sb is None:
                            o_sb = work.tile(
                                [128, 1024], bf16, tag="o_sb", name="o_sb"
                            )
                        nc.vector.tensor_add(o_sb[:, ts(cc, 512)], ps, a_sl)
                if o_sb is not None:  # one store per 128-row band
                    nc.sync.dma_start(out=out.ap()[ts(tt, 128), :], in_=o_sb)
    finally:
        projpool.__exit__(None, None, None)


def _host_inputs(x, w_attn, b_attn, w_proj, b_proj):
    """Build the 8 per-core input maps."""
    x = np.asarray(x, np.float32)
    w_attn = np.asarray(w_attn, np.float32)
    b_attn = np.asarray(b_attn, np.float32)
    w_proj = np.asarray(w_proj, np.float32)
    b_proj = np.asarray(b_proj, np.float32)

    # rope tables, transposed [d, t], full height with rotate-half signs folded:
    # out = x * cos2 + swap_halves(x) * sin2,  cos2=[cos;cos], sin2=[-sin;sin]
    inv_freq = 1.0 / (ROPE_BASE ** (np.arange(0, D, 2, dtype=np.float32) / D))
    freqs = np.arange(T, dtype=np.float32)[:, None] * inv_freq[None, :]  # [T, 64]
    c_ = np.ascontiguousarray(np.cos(freqs).T)  # [64, T]
    s_ = np.ascontiguousarray(np.sin(freqs).T)
    cosT = np.concatenate([c_, c_], axis=0).astype(np.float32)  # [128, T]
    sinT = np.concatenate([-s_, s_], axis=0).astype(np.float32)

    # single lower-triangular [k, q] mask for the diagonal 128x128 block
    k_idx = np.arange(128)
    maskT = (k_idx[:, None] <= k_idx[None, :]).astype(np.float32)

    permM = np.zeros((128, 128), np.float32)
    permM[(np.arange(128) + 64) % 128, np.arange(128)] = 1.0

    in_maps = []
    for c in range(N_CORES):
        b, g = divmod(c, 2)
        cs = slice(g * 1024, (g + 1) * 1024)
        wq = w_attn[:, 0:C][:, cs]
        wk = w_attn[:, C : 2 * C][:, cs]
        wv = w_attn[:, 2 * C : 3 * C][:, cs]
        bq = b_attn[0:C][cs]
        bk = b_attn[C : 2 * C][cs]
        bvv = b_attn[2 * C : 3 * C][cs]
        bqk = np.ascontiguousarray(np.concatenate([bq, bk]).reshape(16, 128).T)
        pk1 = np.concatenate([cosT, sinT, permM], axis=1)  # [128, 2176]
        pk2 = np.concatenate(
            [
                maskT,
                np.broadcast_to(bvv.reshape(1, 1024), (128, 1024)),
                np.broadcast_to(b_proj[cs].reshape(1, 1024), (128, 1024)),
            ],
            axis=1,
        )  # [128, 4096]
        in_maps.append(
            {
                "xT": np.ascontiguousarray(x[b].T).astype(BF16),
                "wqkv": np.concatenate([wq, wk, wv], axis=1).astype(BF16),
                "bqk": bqk.astype(np.float32),
                "pk1": pk1.astype(BF16),
                "pk2": np.ascontiguousarray(pk2).astype(BF16),
                "wproj": w_proj[:, cs].astype(BF16),
            }
        )
    return in_maps


def kernel(x, w_attn, b_attn, w_proj, b_proj, _trace=False):
    from concourse.bass_utils import run_bass_kernel_spmd

    if "nc" not in _PROGRAM_CACHE:
        _PROGRAM_CACHE["nc"] = _build_program()
    nc = _PROGRAM_CACHE["nc"]

    in_maps = _host_inputs(x, w_attn, b_attn, w_proj, b_proj)
    res = run_bass_kernel_spmd(
        nc, in_maps, core_ids=list(range(N_CORES)), trace=_trace
    )
    _PROGRAM_CACHE["last_results"] = res

    out = np.zeros((B, T, C), np.float32)
    for c in range(N_CORES):
        b, g = divmod(c, 2)
        out[b, :, g * 1024 : (g + 1) * 1024] = res.results[c]["out"]
    return out
